# revision 1
# baseline (speedup 1.0000x reference)
"""Trainium2 Bass kernel for nn_Encoder_block (B=128,S=512,D=24,H=4,HD=6,DFF=48).

Strategy: pure data parallel over batch — 16 batches per NeuronCore x 8 cores.
Per core everything runs in "T-layout" ([d, token] with d on partitions),
processed in 4 groups of 4 batches banded onto the 128 partitions
(batch p of a group occupies partitions 32p..32p+24).

v3 design:
  - Every matmul runs at 1 cycle/row: the whole matmul-input chain is bf16
    (weights, x, Q/K staging, attention, LN/FFN activations); accumulation
    and the final LN output stay fp32.
  - Q/K projections write one 2-bank PSUM tile; ONE [128,1024] ACT copy
    stages them to SBUF.  Score chunks are computed in [128,1024] 2-bank
    pairs and drained by ONE exp per pair: ACT true exp or a DVE bit-trick
    (int16(x*A+B) bitcast as bf16 ~ exp; ripple cancels in softmax).
  - Softmax 1/denom partition-broadcast is a single SBUF->SBUF DMA issued
    from the Pool queue (SWDGE); LN rstd broadcast is a selector matmul.
  - GPSIMD/Pool never touches PSUM (illegal on TRN2): it handles SBUF-only
    work (Vsb memset, LN squares, FFN residual add).
  - Emission is software-pipelined: next-batch QKV + V-transposes and the
    previous group's LN/FFN tail are injected into the score stream.
"""

import os
import sys

import numpy as np

for _p in ("/opt/trn_rl_repo", "/opt/trn_rl_repo/concourse"):
    if os.path.isdir(_p) and _p not in sys.path:
        sys.path.insert(0, _p)

import concourse.bass as bass
import concourse.bacc as bacc
import concourse.mybir as mybir
import concourse.tile as tile
from concourse.bass_utils import run_bass_kernel_spmd

F32 = mybir.dt.float32
F32R = mybir.dt.float32r
BF16 = mybir.dt.bfloat16
I16 = mybir.dt.int16
AF = mybir.ActivationFunctionType
ALU = mybir.AluOpType

B, S, D = 128, 512, 24
H, HD, DFF = 4, 6, 48
EPS = 1e-5
NCORES = 8
NB = B // NCORES          # batches per core = 16
SCALE = 1.0 / np.sqrt(HD)  # folded into the exp

# bit-trick exp: int16(round(x*SCALE*128*log2e + 128*127)) bitcast as bf16
# approximates exp(x*SCALE) within [x0.997, x1.061]; the common factor
# cancels between softmax numerator and denominator.
EXP_A = float(128.0 * np.log2(np.e) * SCALE)
EXP_B = float(128.0 * 127.0)

# engine for each of the 8 score-pair exp drains per batch (A=ACT true exp,
# D=DVE bit-trick); two patterns alternated by batch parity for balance.
EXP_PAIR_ENGINES = ["ADADADAD", "ADADADAD"]


def _host_consts(Wq, Wk, Wv, Wo, W1, W2, g1, b1, g2, b2):
    """Pre-layout all weights on the host (numpy) into the banded SBUF forms
    the kernel wants.  All float32."""
    c = {}
    # mm1 lhsT (per band replicated): cols 32h+j (j<6) = Wq[6h+j, :],
    # cols 8..32 = Wv rows; result rows are Q^T bands + V^T block.
    wqk1 = np.zeros((D, 128), np.float32)
    wk2 = np.zeros((D, 128), np.float32)
    for h in range(H):
        for j in range(HD):
            wqk1[:, 32 * h + j] = Wq[6 * h + j, :]
            wk2[:, 32 * h + j] = Wk[6 * h + j, :]
    for dv in range(D):
        wqk1[:, 8 + dv] = Wv[dv, :]
    WQK1 = np.zeros((128, 128), np.float32)
    WK2 = np.zeros((128, 128), np.float32)
    for p in range(4):
        WQK1[32 * p : 32 * p + D, :] = wqk1
        WK2[32 * p : 32 * p + D, :] = wk2
    c["wqk1"] = WQK1
    c["wk2"] = WK2

    # Wo lhsT: rows 32h+1+j = Wo[:, 6h+j]
    WOE = np.zeros((128, 32), np.float32)
    for h in range(H):
        for j in range(HD):
            WOE[32 * h + 1 + j, 0:D] = Wo[:, 6 * h + j]
    c["woe"] = WOE

    # softmax 1/denom broadcast: RB[32h+r] = RR[32h] for r=1..6
    SELR = np.zeros((128, 128), np.float32)
    for h in range(H):
        for r in range(1, 7):
            SELR[32 * h, 32 * h + r] = 1.0
    c["selr"] = SELR

    # LN centering: YC[32p+i] = Y[32p+i] - mean(band p) for i<24, 0 for junk
    CBC = np.zeros((128, 128), np.float32)
    for p in range(4):
        for i in range(D):
            CBC[32 * p + i, 32 * p + i] = 1.0
            CBC[32 * p : 32 * p + D, 32 * p + i] -= 1.0 / D
    c["cbc"] = CBC

    # var: VPS[32p] = mean(band p of YC^2)
    CB2 = np.zeros((128, 128), np.float32)
    for p in range(4):
        CB2[32 * p : 32 * p + D, 32 * p] = 1.0 / D
    c["cb2"] = CB2

    # rstd broadcast with gain folded: RSB[32p+i] = g[i]*RST[32p]
    SELG1 = np.zeros((128, 128), np.float32)
    SELG2 = np.zeros((128, 128), np.float32)
    for p in range(4):
        for i in range(D):
            SELG1[32 * p, 32 * p + i] = g1[i]
            SELG2[32 * p, 32 * p + i] = g2[i]
    c["selg1"] = SELG1
    c["selg2"] = SELG2

    # FFN W1 lhsT: variant p picks band p: rows 32p+d, col 64p+m = W1[m, d]
    W1E = np.zeros((128, 4 * 64), np.float32)
    for p in range(4):
        W1E[32 * p : 32 * p + D, 64 * p : 64 * p + DFF] = W1.T
    c["w1e"] = W1E

    # FFN W2 lhsT: even variant rows 0:48, odd variant rows 64:112
    W2E = np.zeros((128, 2 * 32), np.float32)
    W2E[0:DFF, 0:D] = W2.T
    W2E[64 : 64 + DFF, 32 : 32 + D] = W2.T
    c["w2e"] = W2E

    # identity for PE transposes
    c["idt32"] = np.eye(32, dtype=np.float32)

    # per-partition bias columns [128, 2] = b1, b2 (usually all-zero)
    BB = np.zeros((128, 2), np.float32)
    for p in range(4):
        BB[32 * p : 32 * p + D, 0] = b1
        BB[32 * p : 32 * p + D, 1] = b2
    c["bb"] = BB
    return c


CONST_SHAPES = {
    "wqk1": (128, 128),
    "wk2": (128, 128),
    "woe": (128, 32),
    "selr": (128, 128),
    "cbc": (128, 128),
    "cb2": (128, 128),
    "selg1": (128, 128),
    "selg2": (128, 128),
    "w1e": (128, 4 * 64),
    "w2e": (128, 2 * 32),
    "idt32": (32, 32),
    "bb": (128, 2),
}

# all constants ride in one [128, CONST_COLS] fp32 tensor -> single DMA
CONST_OFFS = {}
_off = 0
for _k, _sh in CONST_SHAPES.items():
    CONST_OFFS[_k] = _off
    _off += _sh[1]
CONST_COLS = _off


def _pack_consts(consts: dict) -> np.ndarray:
    import ml_dtypes

    packed = np.zeros((128, CONST_COLS), ml_dtypes.bfloat16)
    for k, sh in CONST_SHAPES.items():
        packed[: sh[0], CONST_OFFS[k] : CONST_OFFS[k] + sh[1]] = consts[k].astype(
            ml_dtypes.bfloat16
        )
    return packed


def _pin_act_tables():
    """Force Exp and Ln to resolve to the combined natural_log_exp_and_others
    table set (otherwise the compiler ping-pongs exp_and_others <-> natural_log
    at every LayerNorm, ~1.3us per reload)."""
    import concourse.bacc as _bacc
    if getattr(_bacc, "_act_tables_pinned", False):
        return
    _orig = _bacc.get_activation_tables

    def _patched(arch):
        tables = dict(_orig(arch))
        keep = "natural_log_exp_and_others"
        # every ACT function this kernel uses (Exp, Ln, Copy, Square, Relu)
        # lives in the keep set; empty all others so the compiler can never
        # ping-pong table sets mid-kernel (~2.7us per reload).
        for name in list(tables):
            if name != keep:
                tables[name] = set()
        return tables

    _bacc.get_activation_tables = _patched
    _bacc._act_tables_pinned = True


def build_nc(nb: int = NB, use_bias: bool = False) -> bass.Bass:
    """Build the per-core Bass program. nb = batches this core processes."""
    _pin_act_tables()
    ngroups = nb // 4
    nc = bacc.Bacc()
    # host-packed block layout: x[g, 32b+c, f, j] = x_orig[4g+b, 32f+c, j]
    # (j<24 data, j>=24 zero padding so DMAs are fully contiguous)
    x_in = nc.dram_tensor("x", [ngroups, 128, 16, 32], BF16, kind="ExternalInput")
    out = nc.dram_tensor("out", [ngroups, 128, 16, 32], F32, kind="ExternalOutput")
    cpack = nc.dram_tensor("cpack", [128, CONST_COLS], BF16, kind="ExternalInput")

    with tile.TileContext(nc) as tc:
        import contextlib

        ctx = contextlib.ExitStack()
        with ctx:
            constp = ctx.enter_context(tc.tile_pool(name="consts", bufs=1))
            xnp = ctx.enter_context(tc.tile_pool(name="xn", bufs=2))
            xtp = ctx.enter_context(tc.tile_pool(name="xt", bufs=2))
            qkp = ctx.enter_context(tc.tile_pool(name="qk", bufs=3))
            vsbp = ctx.enter_context(tc.tile_pool(name="vsb", bufs=2))
            ep = ctx.enter_context(tc.tile_pool(name="e", bufs=3))
            rrp = ctx.enter_context(tc.tile_pool(name="rr", bufs=2))
            rbp = ctx.enter_context(tc.tile_pool(name="rb", bufs=2))
            otp = ctx.enter_context(tc.tile_pool(name="ot", bufs=2))
            y1p = ctx.enter_context(tc.tile_pool(name="y1", bufs=2))
            x1p = ctx.enter_context(tc.tile_pool(name="x1", bufs=2))
            hsp = ctx.enter_context(tc.tile_pool(name="hs", bufs=2))
            fsp = ctx.enter_context(tc.tile_pool(name="fs", bufs=2))
            ysqp = ctx.enter_context(tc.tile_pool(name="ysq", bufs=2))
            smp = ctx.enter_context(tc.tile_pool(name="sm", bufs=2))
            ytp = ctx.enter_context(tc.tile_pool(name="yt", bufs=2))
            # PSUM: sc2 (3x 2-bank) + uo (1) + wo (1) = 8 banks
            sc2p = ctx.enter_context(tc.tile_pool(name="sc2", bufs=3, space="PSUM"))
            uop = ctx.enter_context(tc.tile_pool(name="uo", bufs=1, space="PSUM"))
            wop = ctx.enter_context(tc.tile_pool(name="wo", bufs=1, space="PSUM"))

            # prefetch group-0 x before the constant DMAs so its
            # descriptor generation doesn't queue behind them on SP
            U0 = xnp.tile([128, 16, 32], BF16, name="xu")
            nc.sync.dma_start(out=U0[:, :, :], in_=x_in[0])

            # ---- load constants (two DMAs: hot weights first) ----
            call = constp.tile([128, CONST_COLS], BF16, name="c_all")
            nc.sync.dma_start(out=call[:, 0:256], in_=cpack[:, 0:256])
            nc.sync.dma_start(
                out=call[:, 256:CONST_COLS], in_=cpack[:, 256:CONST_COLS]
            )
            C = {
                k: call[: sh[0], CONST_OFFS[k] : CONST_OFFS[k] + sh[1]]
                for k, sh in CONST_SHAPES.items()
            }
            eps_t = constp.tile([128, 1], F32, name="c_eps")
            nc.vector.memset(eps_t, EPS)
            if use_bias:
                # biases need an fp32 per-partition scalar operand (the bf16
                # const pack is rejected by tensor_scalar add)
                bbf_in = nc.dram_tensor("bbf", [128, 2], F32, kind="ExternalInput")
                bbf = constp.tile([128, 2], F32, name="c_bbf")
                nc.sync.dma_start(out=bbf, in_=bbf_in[:, :])
            else:
                bbf = None

            st = {}  # live tiles keyed by batch/group index

            def load(g):
                if g == 0:
                    U = U0
                else:
                    U = xnp.tile([128, 16, 32], BF16, name="xu")
                    nc.sync.dma_start(out=U[:, :, :], in_=x_in[g])
                XT4 = xtp.tile([128, S], BF16, name="xt4")
                nc.vector.transpose(XT4[:, :], U.rearrange("p a b -> p (a b)"))
                st[("xt4", g)] = XT4

            def stage1(i):
                """QKV projections for batch i into one 2-bank psum tile +
                a single [128,1024] ACT copy to SBUF."""
                gi, p = divmod(i, 4)
                XT4 = st[("xt4", gi)]
                ps12 = sc2p.tile([128, 2 * S], F32, name="ps12", tag="sc")
                nc.tensor.matmul(
                    ps12[:, 0:S],
                    C["wqk1"][32 * p : 32 * p + D, :],
                    XT4[32 * p : 32 * p + D, :],
                    start=True, stop=True, tile_position=(32 * p, 0),
                )
                nc.tensor.matmul(
                    ps12[:, S : 2 * S],
                    C["wk2"][32 * p : 32 * p + D, :],
                    XT4[32 * p : 32 * p + D, :],
                    start=True, stop=True, tile_position=(32 * p, 0),
                )
                QK = qkp.tile([128, 2 * S], BF16, name="qk")
                nc.scalar.activation(QK[:, :], ps12[:, :], AF.Copy)
                st[("qk", i)] = QK

            def stage2(i):
                """V^T -> [token, d] transposes + banded V lhsT for batch i."""
                QK = st[("qk", i)]
                psv = sc2p.tile([128, 4 * 32], BF16, name="psv", tag="sc")
                for t in range(4):
                    nc.tensor.transpose(
                        psv[:, 32 * t : 32 * (t + 1)],
                        QK[0:32, 128 * t : 128 * (t + 1)],
                        C["idt32"][:, :],
                    )
                Vsb = vsbp.tile([128, 4, 4, 32], BF16, name="vsb")
                nc.gpsimd.memset(Vsb[:, :, :, :], 1.0)
                nc.vector.tensor_copy(
                    Vsb[:, :, :, 1:7],
                    psv.rearrange("p (t x) -> p t x", t=4)[:, :, 8:32].rearrange(
                        "p t (h d) -> p t h d", d=6
                    ),
                )
                st[("vsb", i)] = Vsb

            def scores_pair(i, t, half):
                """One [128,1024] 2-bank pair of score chunks (heads 2*half,
                2*half+1) + its exp drain."""
                QK = st[("qk", i)]
                E = st[("e", i)]
                stt = sc2p.tile([128, 2 * S], F32, name="stt", tag="sc")
                for j in range(2):
                    h = 2 * half + j
                    nc.tensor.matmul(
                        stt[:, j * S : (j + 1) * S],
                        QK[32 * h : 32 * h + HD, S + 128 * t : S + 128 * (t + 1)],
                        QK[32 * h : 32 * h + HD, 0:S],
                        start=True, stop=True,
                        tile_position=(32 * h, 0),
                        skip_group_check=True,
                    )
                eng = EXP_PAIR_ENGINES[i % 2][2 * t + half]
                dst = E[:, t, 2 * half : 2 * half + 2, :]
                if eng == "A":
                    nc.scalar.activation(
                        dst.rearrange("p a q -> p (a q)"), stt[:, :],
                        AF.Exp, scale=float(SCALE),
                    )
                else:
                    nc.vector.tensor_scalar(
                        dst.rearrange("p a q -> p (a q)").bitcast(I16),
                        stt[:, :], EXP_A, EXP_B,
                        op0=ALU.mult, op1=ALU.add,
                    )

            def avs(i, t):
                E = st[("e", i)]
                Vsb = st[("vsb", i)]
                UO = st[("uo", i)]
                for h in range(4):
                    nc.tensor.matmul(
                        UO[32 * h : 32 * h + 32, :],
                        Vsb[:, t, h, :],
                        E[:, t, h, :],
                        start=(t == 0), stop=(t == 3),
                        tile_position=(0, 32 * h),
                        skip_group_check=True,
                    )

            def finalize(i):
                """softmax normalize + Wo for batch i (deferred one batch)."""
                gi, p = divmod(i, 4)
                UOs = st.pop(("uos", i))
                RRb = st.pop(("rrb", i))
                if ("wops", gi) not in st:
                    st[("wops", gi)] = wop.tile([128, S], F32, name="wops")
                WOPS = st[("wops", gi)]
                RB = sc2p.tile([128, 2 * S], F32, name="rbps", tag="sc")
                nc.tensor.matmul(
                    RB[:, 0:S], C["selr"][:, :], RRb[:, :],
                    start=True, stop=True, tile_position=(0, 0),
                    skip_group_check=True,
                )
                OTn = otp.tile([128, S], BF16, name="otn")
                nc.vector.tensor_tensor(OTn[:, :], UOs[:, :], RB[:, 0:S], op=ALU.mult)
                nc.tensor.matmul(
                    WOPS[32 * p : 32 * p + 32, :],
                    C["woe"][:, :],
                    OTn[:, :],
                    start=True, stop=True, tile_position=(0, 32 * p),
                    skip_group_check=True,
                )
                if p == 3:
                    # group residual: emitted here so the single WOPS psum
                    # bank is freed before the next group's first wo matmul.
                    XT4 = st.pop(("xt4", gi))
                    WOPS = st.pop(("wops", gi))
                    Y1 = y1p.tile([128, S], BF16, name="y1")
                    nc.vector.tensor_tensor(
                        Y1[:, :], WOPS[:, :], XT4[:, :], op=ALU.add
                    )
                    st[("y1", gi)] = Y1

            def ln_segs(g, Y, selg, bcol, OUT, lab, fast=False):
                """LayerNorm over d of Y [128,512] (SBUF f32r) as emission
                segments.  YC and VPS share one 2-bank psum tile.  fast=True
                (final group) trades psum-bank hold time for chain latency:
                no ACT staging copy, square on DVE straight from psum."""
                d = {}

                def s1():  # centered Y
                    d["yv"] = sc2p.tile([128, 2 * S], F32, name=f"yv{lab}", tag="sc")
                    nc.tensor.matmul(
                        d["yv"][:, 0:S], C["cbc"][:, :], Y[:, :],
                        start=True, stop=True, tile_position=(0, 0),
                        skip_group_check=True,
                    )

                def s2():  # stage centered Y out of PSUM (frees the bank
                    # pair and dodges the 1-PSUM-operand rule downstream).
                    # fast mode: ycs on DVE + square on ACT, in parallel.
                    d["ycs"] = smp.tile([128, S], F32, name=f"ycs{lab}", tag="ycs")
                    if fast:
                        nc.vector.tensor_copy(d["ycs"][:, :], d["yv"][:, 0:S])
                        d["ysq"] = ysqp.tile([128, S], BF16, name=f"ysq{lab}", tag="ysq")
                        nc.scalar.activation(
                            d["ysq"][:, :], d["yv"][:, 0:S], AF.Square
                        )
                    else:
                        nc.scalar.activation(d["ycs"][:, :], d["yv"][:, 0:S], AF.Copy)

                def s3():  # square (SBUF-only, Pool) + var matmul into bank 2
                    if fast:
                        YSQ = d["ysq"]
                    else:
                        YSQ = ysqp.tile([128, S], BF16, name=f"ysq{lab}", tag="ysq")
                        nc.gpsimd.tensor_mul(YSQ[:, :], d["ycs"][:, :], d["ycs"][:, :])
                    nc.tensor.matmul(
                        d["yv"][:, S : 2 * S], C["cb2"][:, :], YSQ[:, :],
                        start=True, stop=True, tile_position=(0, 0),
                        skip_group_check=True,
                    )

                def s4():  # rstd = exp(-0.5*ln(var+eps))
                    LNV = smp.tile([128, S], F32, name=f"lnv{lab}", tag="sm")
                    nc.scalar.activation(
                        LNV[:, :], d["yv"][:, S : 2 * S], AF.Ln, bias=eps_t[:, :]
                    )
                    d["rst"] = smp.tile([128, S], BF16, name=f"rst{lab}", tag="sm2")
                    nc.scalar.activation(d["rst"][:, :], LNV[:, :], AF.Exp, scale=-0.5)

                def s5():  # gain-folded broadcast + final normalize
                    RSB = sc2p.tile([128, 2 * S], F32, name=f"rsb{lab}", tag="sc")
                    nc.tensor.matmul(
                        RSB[:, 0:S], selg[:, :], d["rst"][:, :],
                        start=True, stop=True, tile_position=(0, 0),
                        skip_group_check=True,
                    )
                    nc.vector.tensor_tensor(
                        OUT[:, :], d["ycs"][:, :], RSB[:, 0:S], op=ALU.mult
                    )
                    if use_bias:
                        nc.vector.tensor_scalar(
                            OUT[:, :], OUT[:, :], bcol, None,
                            op0=ALU.add,
                        )

                return [s1, s2, s3, s4, s5]

            def tail_segs(g):
                """LN1+FFN+LN2+store for group g as segments (Y1 was already
                emitted by finalize of the group's last batch)."""
                fast = g == ngroups - 1
                Y1 = st.pop(("y1", g))
                X1 = x1p.tile([128, S], BF16, name="x1", tag="x1")
                FS = fsp.tile([128, S], BF16, name="fs", tag="fs")
                Y2N = x1p.tile([128, S], F32, name="y2n", tag="x1b")
                segs = []
                segs += ln_segs(g, Y1, C["selg1"],
                                bbf[:, 0:1] if use_bias else None, X1, f"a{g}", fast)
                d = {}

                def f1():
                    d["hs"] = []
                    hps = sc2p.tile([128, 2 * S], F32, name="hps", tag="sc")
                    for p in range(4):
                        # band-sliced lhsT/rhs in distinct 32-row groups so
                        # all four matmuls can pack the PE array concurrently
                        nc.tensor.matmul(
                            hps[64 * (p % 2) : 64 * (p % 2) + 64,
                                (p // 2) * S : (p // 2 + 1) * S],
                            C["w1e"][32 * p : 32 * p + 32, 64 * p : 64 * (p + 1)],
                            X1[32 * p : 32 * p + 32, :],
                            start=True, stop=True,
                            tile_position=(32 * p, 64 * (p % 2)),
                            skip_group_check=True,
                        )
                    d["hps"] = hps

                def f2():
                    HS = hsp.tile([128, 2 * S], BF16, name="hs", tag="hs")
                    nc.vector.tensor_scalar_max(HS[:, :], d["hps"][:, :], 0.0)
                    d["hs"] = [HS[:, 0:S], HS[:, S : 2 * S]]

                def f3():
                    F4 = uop.tile([128, S], F32, name="f4", tag="uo")
                    for p in range(4):
                        # 64-row-sliced operands: even batches use array rows
                        # 0:64, odd 64:128 -> pairwise concurrency
                        nc.tensor.matmul(
                            F4[32 * p : 32 * p + 32, :],
                            C["w2e"][64 * (p % 2) : 64 * (p % 2) + 64,
                                     32 * (p % 2) : 32 * (p % 2) + 32],
                            d["hs"][p // 2][64 * (p % 2) : 64 * (p % 2) + 64, :],
                            start=True, stop=True,
                            tile_position=(64 * (p % 2), 32 * p),
                            skip_group_check=True,
                        )
                    d["f4"] = F4

                def f4s():
                    FSr = fsp.tile([128, S], F32, name="fsr", tag="fsr")
                    nc.scalar.activation(FSr[:, :], d["f4"][:, :], AF.Relu)
                    if fast:
                        nc.vector.tensor_add(FS[:, :], FSr[:, :], X1[:, :])
                    else:
                        nc.gpsimd.tensor_add(FS[:, :], FSr[:, :], X1[:, :])

                segs += [f1, f2, f3, f4s]
                segs += ln_segs(g, FS, C["selg2"],
                                bbf[:, 1:2] if use_bias else None, Y2N, f"b{g}", fast)

                def t9():
                    Y2T = ytp.tile([128, S], F32, name="y2t")
                    if fast:
                        for u in range(2):
                            sl = slice(u * (S // 2), (u + 1) * (S // 2))
                            nc.vector.transpose(Y2T[:, sl], Y2N[:, sl])
                            nc.sync.dma_start(
                                out=out[g][:, u * 8 : (u + 1) * 8, :],
                                in_=Y2T[:, sl].rearrange("q (f c) -> q f c", c=32),
                            )
                    else:
                        nc.vector.transpose(Y2T[:, :], Y2N[:, :])
                        nc.sync.dma_start(
                            out=out[g],
                            in_=Y2T.rearrange("q (f c) -> q f c", c=32),
                        )

                segs.append(t9)
                return segs

            pending_tail = []

            def pop_tail(n=1):
                for _ in range(n):
                    if pending_tail:
                        pending_tail.pop(0)()

            # ---- prologue ----
            load(0)
            stage1(0)
            stage2(0)

            for i in range(nb):
                gi, p = divmod(i, 4)
                st[("e", i)] = ep.tile([128, 4, 4, S], BF16, name="e")
                st[("uo", i)] = uop.tile([128, S], F32, name="uo", tag="uo")
                for t in range(4):
                    for half in range(2):
                        scores_pair(i, t, half)
                    if t == 0:
                        if p == 2 and gi + 1 < ngroups:
                            load(gi + 1)
                        pop_tail()
                    elif t == 1:
                        if i > 0:
                            finalize(i - 1)
                            if p == 0:
                                # one no-op slot so the tail's first matmul
                                # doesn't enter the PE queue before Y1's
                                # normalize->wo chain can finish
                                pending_tail.extend([lambda: None] * 1)
                                pending_tail.extend(tail_segs(gi - 1))
                        if i + 1 < nb:
                            stage1(i + 1)
                        pop_tail()
                    elif t == 2:
                        pop_tail()
                    else:
                        pop_tail()
                    if t > 0:
                        avs(i, t - 1)
                avs(i, 3)
                UO = st.pop(("uo", i))
                RR = rrp.tile([128, S], F32, name="rrt")
                nc.vector.reciprocal_approx_fast(RR[:, :], UO[:, :])
                RRb = rbp.tile([128, S], BF16, name="rrb")
                nc.vector.tensor_copy(RRb[:, :], RR[:, :])
                UOs = otp.tile([128, S], BF16, name="uos", tag="uos")
                nc.scalar.activation(UOs[:, :], UO[:, :], AF.Copy)
                st[("uos", i)] = UOs
                st[("rrb", i)] = RRb
                if i + 1 < nb:
                    stage2(i + 1)

            # ---- epilogue ----
            finalize(nb - 1)
            pending_tail.extend(tail_segs(ngroups - 1))
            pop_tail(len(pending_tail))
    nc.compile()
    return nc


_NC_CACHE: dict = {}


def _get_nc(nb: int, use_bias: bool = False) -> bass.Bass:
    key = (nb, use_bias)
    if key not in _NC_CACHE:
        _NC_CACHE[key] = build_nc(nb, use_bias)
    return _NC_CACHE[key]


def kernel(x, Wq, Wk, Wv, Wo, W1, W2, g1, b1, g2, b2):
    x = np.asarray(x, np.float32)
    args = [np.asarray(a, np.float32) for a in (Wq, Wk, Wv, Wo, W1, W2, g1, b1, g2, b2)]
    consts = _host_consts(*args)
    use_bias = bool(np.any(args[7]) or np.any(args[9]))
    nc = _get_nc(NB, use_bias)
    ngroups = NB // 4
    in_maps = []
    for c in range(NCORES):
        xs = x[c * NB : (c + 1) * NB]
        # pack: xp[g, 32b+cc, f, j<24] = xs[4g+b, 32f+cc, j] (bf16)
        import ml_dtypes

        xp = np.zeros((ngroups, 128, 16, 32), ml_dtypes.bfloat16)
        xp[..., :D] = (
            xs.reshape(ngroups, 4, 16, 32, D)
            .transpose(0, 1, 3, 2, 4)
            .reshape(ngroups, 128, 16, D)
            .astype(ml_dtypes.bfloat16)
        )
        m = {"x": xp, "cpack": _pack_consts(consts)}
        if use_bias:
            m["bbf"] = consts["bb"]
        in_maps.append(m)
    res = run_bass_kernel_spmd(nc, in_maps, list(range(NCORES)))
    outs = []
    for r in res.results:
        op = r["out"]  # [g, 32b+r, f, j]
        o = (
            op[..., :D]
            .reshape(ngroups, 4, 32, 16, D)
            .transpose(0, 1, 3, 2, 4)
            .reshape(NB, S, D)
        )
        outs.append(o)
    return np.concatenate(outs, axis=0)

